# revision 45
# baseline (speedup 1.0000x reference)
"""Trainium2 Bass kernel for nn_Block_17033840296551 (GNN message passing block).

Data-parallel over batch: 16 images -> 8 cores x 2 images. Software-pipelined
emission F0 F1 G0 B0 G1 B1 so the two images overlap (F = g1/norms/sim/topk/
q/p/idx-repack, G = gather+maxfold, B = g2/FFN/bottleneck/out).

Key points vs the straightforward version:
  * wrapped-idx columns laid out (g, k, i) per s-block so the top-k index
    repack is 8 contiguous per-g DMAs + 3 replication DMAs instead of ~39
    scattered ones; the gather output column order is reconciled with node
    order by strided eT/pT views in the fold tail (i-split, 3 free dims).
  * All weights in 3 DRAM params (f16 blob / f32 biases / skeleton) = 3 DMAs.
  * sim scores stay in PSUM; DVE Max/MaxIndex read PSUM directly (no evac).
  * EdgeConv gather runs from SBUF (q never round-trips DRAM).
  * PSUM evacuations are single fused ops (activation w/ bias, tensor_scalar
    relu(x+b), scalar_tensor_tensor (x+b)+res) spread across Act/DVE/Pool.
  * f16 residual stream; f32 only in PSUM and the final output.
"""

import os
import numpy as np

B, C, H, W = 16, 256, 32, 32
N = H * W
K = 9
EPS = 1e-5
IMGS_PER_CORE = 2
N_CORES = 8
NEG_BIG = -30000.0

_cache = {}

# f16 blob column offsets (per partition, f16 words)
_F16_OFF = {
    'wg1': (0, 512), 'wp': (512, 1024), 'wq': (1536, 1024), 'wg2': (2560, 1024),
    'wf1': (3584, 2048), 'wf2': (5632, 2048), 'wb1': (7680, 128),
    'wb2': (7808, 576), 'wb3': (8384, 256), 'negid': (8640, 128),
    'idbig2': (8768, 2048),
}
F16W = 10816
# f32 param columns
_F32_OFF = {'bt1': (0, 2), 'bt2': (2, 2), 'bbp': (4, 4), 'bbf1': (8, 8),
            'bbf2': (16, 2), 'btb3': (18, 2), 'bsf': (20, 2), 'btf': (22, 2),
            'btb1': (24, 1), 'btb2': (25, 1), 'nbbp': (26, 4),
            'b3b': (30, 2)}
F32W = 32


def _bn_fold(p):
    g, b, m, v = np.asarray(p, np.float32)
    s = g / np.sqrt(v + EPS)
    t = b - m * s
    return s, t


def _pack_kxm(w_t, part=128):
    Kd, M = w_t.shape
    kt = Kd // part
    return np.ascontiguousarray(w_t.reshape(kt, part, M).transpose(1, 0, 2))


def _pack_bias(b, part=128):
    n = b.shape[0]
    t = n // part
    return np.ascontiguousarray(b.reshape(t, part).T)  # [part, t]


def _make_skeleton():
    """wrapped-idx self columns: col = 144*s + 18*g + 2*0 + i holds node
    n_self = 256*s + 128*i + 16*g + p16 on partition p16 (the node whose
    neighbor list lives at ixbuf[16*g + p16, s, :, i])."""
    skel = np.zeros((16, 576), np.int16)
    for s in range(4):
        for g in range(8):
            for i in range(2):
                col = 144 * s + 18 * g + i
                for p16 in range(16):
                    skel[p16, col] = 256 * s + 128 * i + 16 * g + p16
    return skel


def _prep_weights(inp):
    f16 = np.float16
    s1, t1 = _bn_fold(inp['g1_bn'])
    Wg1 = s1[:, None] * inp['g1_w']
    s2, t2 = _bn_fold(inp['g2_bn'])
    Wg2 = s2[:, None] * inp['g2_w']
    sf1, tf1 = _bn_fold(inp['f1_bn'])
    Wf1 = sf1[:, None] * inp['f1_w']
    bf1 = sf1 * inp['f1_b'] + tf1
    sf2, tf2 = _bn_fold(inp['f2_bn'])
    Wf2 = sf2[:, None] * inp['f2_w']
    bf2 = sf2 * inp['f2_b'] + tf2
    sb1, tb1 = _bn_fold(inp['b1_bn'])
    Wb1 = sb1[:, None] * inp['b1_w']
    sb2, tb2 = _bn_fold(inp['b2_bn'])
    Wb2 = sb2[:, None, None, None] * inp['b2_w']
    sb3, tb3 = _bn_fold(inp['b3_bn'])
    Wb3 = sb3[:, None] * inp['b3_w']
    sf, tf = _bn_fold(inp['bnf'])

    A = inp['edge_w'][:, :C]
    Bm = inp['edge_w'][:, C:]
    Wp = A - Bm
    Wq = Bm
    bp = inp['edge_b']

    wb2_t = np.zeros((64, 9, 64), f16)
    for dy in range(3):
        for dx in range(3):
            wb2_t[:, dy * 3 + dx, :] = Wb2[:, :, dy, dx].T.astype(f16)

    f16b = np.zeros((128, F16W), f16)

    def put16(name, arr):
        off, n = _F16_OFF[name]
        p = arr.shape[0]
        f16b[:p, off:off + n] = arr.reshape(p, -1)

    put16('wg1', _pack_kxm(Wg1.T.astype(f16)))
    put16('wp', _pack_kxm(Wp.T.astype(f16)))
    put16('wq', _pack_kxm(Wq.T.astype(f16)))
    put16('wg2', _pack_kxm(Wg2.T.astype(f16)))
    put16('wf1', _pack_kxm(Wf1.T.astype(f16)))
    put16('wf2', _pack_kxm(Wf2.T.astype(f16)))
    put16('wb1', _pack_kxm(Wb1.T.astype(f16)))
    put16('wb2', wb2_t)
    put16('wb3', Wb3.T.astype(f16))
    put16('negid', (NEG_BIG * np.eye(128)).astype(f16))
    idbig2 = np.zeros((128, 2048), f16)
    for k in range(128):
        idbig2[k, 1024 + k] = 1.0
    put16('idbig2', idbig2)

    f32b = np.zeros((128, F32W), np.float32)

    def put32(name, arr):
        off, n = _F32_OFF[name]
        p = arr.shape[0]
        f32b[:p, off:off + n] = arr.reshape(p, -1)

    put32('bt1', _pack_bias(t1))
    put32('bt2', _pack_bias(t2))
    put32('bbp', _pack_bias(bp))
    put32('nbbp', _pack_bias(-bp))
    put32('bbf1', _pack_bias(bf1))
    put32('bbf2', _pack_bias(bf2))
    put32('btb3', _pack_bias(tb3))
    put32('bsf', _pack_bias(sf))
    put32('btf', _pack_bias(tf))
    put32('btb1', tb1[:, None].astype(np.float32))
    put32('btb2', tb2[:, None].astype(np.float32))
    put32('b3b', _pack_bias(sf * tb3 + tf))

    return {'wf16': f16b, 'wf32': f32b, 'skel': _make_skeleton()}


def _build_bass():
    import concourse.bass as bass
    import concourse.mybir as mybir
    from concourse import bacc
    from concourse.tile import TileContext

    dt = mybir.dt
    F16 = dt.float16
    F32 = dt.float32
    AF = mybir.ActivationFunctionType
    OP = mybir.AluOpType

    nc = bacc.Bacc()

    x_d = nc.declare_dram_parameter("x", [IMGS_PER_CORE, C, N], F32, isOutput=False)
    wf16_d = nc.declare_dram_parameter("wf16", [128, F16W], F16, isOutput=False)
    wf32_d = nc.declare_dram_parameter("wf32", [128, F32W], F32, isOutput=False)
    skel_d = nc.declare_dram_parameter("skel", [16, 576], dt.int16, isOutput=False)
    out_d = nc.declare_dram_parameter("out", [IMGS_PER_CORE, C, N], F16, isOutput=True)

    with TileContext(nc) as tc:
        import contextlib
        ctx = contextlib.ExitStack()
        with ctx:
            consts = ctx.enter_context(tc.tile_pool(name="consts", bufs=1))
            pool_x = ctx.enter_context(tc.tile_pool(name="x", bufs=2))
            pool_feat = ctx.enter_context(tc.tile_pool(name="feat", bufs=2))
            pool_nrm = ctx.enter_context(tc.tile_pool(name="nrm", bufs=2))
            pool_sm = ctx.enter_context(tc.tile_pool(name="sm", bufs=2))
            pool_mx = ctx.enter_context(tc.tile_pool(name="mx", bufs=4))
            pool_idx = ctx.enter_context(tc.tile_pool(name="idx", bufs=2))
            pool_qp = ctx.enter_context(tc.tile_pool(name="qp", bufs=2))
            pool_e = ctx.enter_context(tc.tile_pool(name="e", bufs=2))
            pool_go = ctx.enter_context(tc.tile_pool(name="go", bufs=4))
            pool_h = ctx.enter_context(tc.tile_pool(name="h", bufs=1))
            pool_h2 = ctx.enter_context(tc.tile_pool(name="h2", bufs=2))
            pool_tmp = ctx.enter_context(tc.tile_pool(name="tmp", bufs=3))
            pool_b = ctx.enter_context(tc.tile_pool(name="b", bufs=1))
            psum = ctx.enter_context(tc.tile_pool(name="psum", bufs=2, space="PSUM"))
            psums = ctx.enter_context(tc.tile_pool(name="psims", bufs=2, space="PSUM"))

            # ---- consts ----
            wf16 = consts.tile([128, F16W], F16, name="wf16")
            wf32 = consts.tile([128, F32W], F32, name="wf32")
            nc.sync.dma_start(out=wf16[:, 0:512], in_=wf16_d[:, 0:512])
            nc.sync.dma_start(out=wf32[:], in_=wf32_d[:])

            def w16(name, *shape):
                off, n = _F16_OFF[name]
                v = wf16[:, off:off + n]
                if len(shape) > 1:
                    v = v.rearrange("p (a b) -> p a b", a=shape[0])
                return v

            wg1 = w16('wg1', 2, 256)
            wp = w16('wp', 2, 512)
            wq = w16('wq', 2, 512)
            wg2 = w16('wg2', 4, 256)
            wf1 = w16('wf1', 2, 1024)
            wf2 = w16('wf2', 8, 256)
            wb1 = w16('wb1', 2, 64)
            wb2 = w16('wb2', 9, 64)
            wb3 = w16('wb3', 256)
            negid = w16('negid', 128)
            idbig2 = w16('idbig2', 2048)

            def b32(name):
                off, n = _F32_OFF[name]
                return wf32[:, off:off + n]

            bt1, bt2, bbp, bbf1 = b32('bt1'), b32('bt2'), b32('bbp'), b32('bbf1')
            bbf2, btb3, bsf, btf = b32('bbf2'), b32('btb3'), b32('bsf'), b32('btf')
            btb1, btb2, b3b = b32('btb1'), b32('btb2'), b32('b3b')

            ones = consts.tile([128, 128], F16, name="ones")
            nc.gpsimd.memset(ones[:], 1.0)
            warm = consts.tile([1, 8], F16, name="warm")
            nc.gpsimd.memset(warm[:], 1.0)
            nc.scalar.activation(out=warm[:], in_=warm[:], func=AF.Identity)
            nc.scalar.activation(out=warm[:], in_=warm[:], func=AF.Sqrt)

            # persistent wrapped-idx tiles (self cols filled once)
            wrapped = [consts.tile([128, 576], dt.int16, name=f"wr{i}")
                       for i in range(IMGS_PER_CORE)]
            for i in range(IMGS_PER_CORE):
                nc.sync.dma_start(out=wrapped[i][0:16, :], in_=skel_d[:])

            xcs, feats, xns, ixbufs, qsbs, pTs, eTs = {}, {}, {}, {}, {}, {}, {}
            houts = {}

            # ================= F phase =================
            def emit_F_front(img):
                xc = pool_x.tile([128, 2, N], F16, name="xc")
                xcs[img] = xc
                nc.gpsimd.dma_start(
                    out=xc[:],
                    in_=x_d[img].rearrange("(t p) n -> p t n", p=128))
                if img == 0:
                    nc.sync.dma_start(out=wf16[:, 512:], in_=wf16_d[:, 512:])

                featT = pool_feat.tile([128, 2, N], F16, name="featT")
                feats[img] = featT
                for to in range(2):
                    ps = psum.tile([128, 1024], F32, name="ps_g1", tag="ps")
                    for nb in range(2):
                        for kt in range(2):
                            nc.tensor.matmul(
                                ps[:, nb * 512:(nb + 1) * 512],
                                lhsT=wg1[:, kt, to * 128:(to + 1) * 128],
                                rhs=xc[:, kt, nb * 512:(nb + 1) * 512],
                                start=(kt == 0), stop=(kt == 1))
                    nc.scalar.activation(out=featT[:, to, :], in_=ps[:],
                                         func=AF.Identity, bias=bt1[:, to:to + 1])

                # norms
                ve = nc.vector
                fsq = pool_nrm.tile([128, 2, N], F16, name="fsq")
                for t in range(2):
                    if img == 0:
                        nc.vector.tensor_mul(fsq[:, t, :], featT[:, t, :],
                                             featT[:, t, :])
                    else:
                        nc.scalar.activation(out=fsq[:, t, :],
                                             in_=featT[:, t, :], func=AF.Square)
                n2 = pool_sm.tile([1, N], F32, name="n2")
                for nb in range(2):
                    ps1 = psum.tile([128, 1024], F32, name="ps_n2", tag="ps")
                    for kt in range(2):
                        nc.tensor.matmul(
                            ps1[0:1, 0:512], lhsT=ones[:, 0:1],
                            rhs=fsq[:, kt, nb * 512:(nb + 1) * 512],
                            start=(kt == 0), stop=(kt == 1))
                    nc.scalar.activation(out=n2[:, nb * 512:(nb + 1) * 512],
                                         in_=ps1[0:1, 0:512], func=AF.Identity)
                nc.vector.reciprocal(out=n2[:], in_=n2[:])
                invn = pool_sm.tile([1, N], F16, name="invn")
                nc.scalar.activation(out=invn[:], in_=n2[:], func=AF.Sqrt)
                invnb = pool_nrm.tile([128, N], F16, name="invnb")
                ps = psum.tile([128, 1024], F32, name="ps_bc", tag="ps")
                for nb in range(2):
                    nc.tensor.matmul(ps[:, nb * 512:(nb + 1) * 512],
                                     lhsT=ones[0:1, :],
                                     rhs=invn[:, nb * 512:(nb + 1) * 512],
                                     start=True, stop=True)
                nc.scalar.activation(out=invnb[:], in_=ps[:], func=AF.Identity)
                xnT = pool_nrm.tile([128, 2, N], F16, name="xnT")
                for t in range(2):
                    ve.tensor_mul(xnT[:, t, :], featT[:, t, :], invnb[:])
                xns[img] = xnT

            def emit_F_main(img, hooks=None):
                featT = feats[img]
                xnT = xns[img]
                # sim + top-8 (Max/MaxIndex read PSUM directly)
                ixbuf = pool_idx.tile([128, 4, 8, 2], dt.uint16, name="ixbuf")
                ixbufs[img] = ixbuf
                for I in range(8):
                    ps = psums.tile([128, 1024], F32, name="ps_sim", tag="psim")
                    for cb in range(2):
                        has_diag = (cb == I // 4)
                        for kt in range(2):
                            nc.tensor.matmul(
                                ps[:, cb * 512:(cb + 1) * 512],
                                lhsT=featT[:, kt, I * 128:(I + 1) * 128],
                                rhs=xnT[:, kt, cb * 512:(cb + 1) * 512],
                                start=(kt == 0),
                                stop=(kt == 1 and not has_diag))
                        if has_diag:
                            Wo = 1024 - 128 * (I % 4)
                            nc.tensor.matmul(
                                ps[:, cb * 512:(cb + 1) * 512],
                                lhsT=negid,
                                rhs=idbig2[:, Wo:Wo + 512],
                                start=False, stop=True)
                    mx = pool_mx.tile([128, 8], F32, name="mx")
                    nc.vector.max(out=mx[:], in_=ps[:])
                    nc.vector.max_index(out=ixbuf[:, I // 2, :, I % 2],
                                        in_max=mx[:], in_values=ps[:])
                    for fn in (hooks or {}).get(I, []):
                        fn()

                # q (pairs of nt into one [128,1024] psum)
                q_sb = pool_qp.tile([128, 8, 512], F16, name="q_sb")
                qsbs[img] = q_sb
                for np_ in range(4):
                    ps = psum.tile([128, 1024], F32, name="ps_q", tag="ps")
                    for half in range(2):
                        nt = 2 * np_ + half
                        for kt in range(2):
                            nc.tensor.matmul(
                                ps[:, half * 512:(half + 1) * 512],
                                lhsT=featT[:, kt, nt * 128:(nt + 1) * 128],
                                rhs=wq[:, kt, :], start=(kt == 0), stop=(kt == 1))
                    nc.scalar.activation(
                        out=q_sb[:].rearrange("p a b -> p (a b)")[:, np_ * 1024:(np_ + 1) * 1024],
                        in_=ps[:], func=AF.Identity)

                # p^T (bias folded at evac)
                pT = pool_qp.tile([128, 4, N], F16, name="pT")
                pTs[img] = pT
                for to in range(4):
                    ps = psum.tile([128, 1024], F32, name="ps_p", tag="ps")
                    for nb in range(2):
                        for kt in range(2):
                            nc.tensor.matmul(
                                ps[:, nb * 512:(nb + 1) * 512],
                                lhsT=wp[:, kt, to * 128:(to + 1) * 128],
                                rhs=featT[:, kt, nb * 512:(nb + 1) * 512],
                                start=(kt == 0), stop=(kt == 1))
                    nc.scalar.activation(out=pT[:, to, :], in_=ps[:],
                                         func=AF.Identity, bias=bbp[:, to:to + 1])

                # idx repack: 8 contiguous fold DMAs + 3 replication DMAs
                wr = wrapped[img]
                wrv = wr.rearrange("p (s c) -> p s c", s=4)
                ixi = ixbuf[:].bitcast(dt.int16)
                for g in range(8):
                    nc.sync.dma_start(
                        out=wrv[0:16, :, 18 * g + 2:18 * g + 18],
                        in_=ixi[16 * g:16 * (g + 1)].rearrange("p s k i -> p s (k i)"))
                nc.sync.dma_start(out=wr[16:32, :], in_=wr[0:16, :])
                nc.sync.dma_start(out=wr[32:64, :], in_=wr[0:32, :])
                nc.sync.dma_start(out=wr[64:128, :], in_=wr[0:64, :])

            # ================= G phase =================
            gos = {}

            def emit_G_start(img):
                eTs[img] = pool_e.tile([128, 4, N], F16, name="eT")

            def gather_s(img, s, h):
                q_sb = qsbs[img]
                wr = wrapped[img]
                go = pool_go.tile([128, 4, 1152], F16, name="go")
                gos[(img, s, h)] = go
                nc.gpsimd.dma_gather(
                    out_ap=go[:], in_ap=q_sb[:].rearrange("p a b -> p (a b)"),
                    idxs_ap=wr[:, 144 * s + 72 * h:144 * s + 72 * (h + 1)],
                    num_idxs=1152, num_idxs_reg=1152, elem_size=512,
                    transpose=True, sbuf_tokens_per_rank=128,
                    sbuf_free_dim_per_rank=1024,
                    single_packet=False)

            def fold_s(img, s, h):
                go = gos.pop((img, s, h))
                pT = pTs[img]
                eT = eTs[img]
                # go cols: j = 288g' + 32k + 16i + p16, g' in 0..3 (g = 4h+g')
                gv = go[:].rearrange("p a (g k w) -> p (a g) k w", g=4, k=9)
                nc.vector.tensor_max(gv[:, :, 5:9, :], gv[:, :, 1:5, :],
                                     gv[:, :, 5:9, :])
                nc.vector.tensor_max(gv[:, :, 7:9, :], gv[:, :, 5:7, :],
                                     gv[:, :, 7:9, :])
                nc.vector.tensor_max(gv[:, :, 8, :], gv[:, :, 7, :],
                                     gv[:, :, 8, :])
                # tail: output (g', i, q) <-> node 256s + 128i + 16(4h+g') + q
                g6 = go[:].rearrange("p a (g k i q) -> p a g k i q",
                                     g=4, k=9, i=2)
                for i in range(2):
                    base = 256 * s + 128 * i + 64 * h
                    ev = eT[:, :, base:base + 64].rearrange(
                        "p a (g q) -> p a g q", g=4)
                    pv = pT[:, :, base:base + 64].rearrange(
                        "p a (g q) -> p a g q", g=4)
                    nc.vector.tensor_max(ev, g6[:, :, :, 8, i, :],
                                         g6[:, :, :, 0, i, :])
                    nc.vector.tensor_add(ev, ev, pv)
                    nc.scalar.activation(out=eT[:, :, base:base + 64],
                                         in_=eT[:, :, base:base + 64],
                                         func=AF.Relu)

            # ================= B phase =================
            def emit_B(img):
                xc = xcs[img]
                eT = eTs[img]
                # g2 + residual -> h f16
                pad = pool_b.tile([64, 34 * 34], F16, name="pad")
                nc.gpsimd.memset(pad[:], 0.0)
                h = pool_h.tile([128, 2, N], F16, name="h")
                for to in range(2):
                    pp = psum if to % 2 == 0 else psums
                    ps = pp.tile([128, 1024], F32, name="ps_g2",
                                 tag="ps" if to % 2 == 0 else "psim")
                    for nb in range(2):
                        for kt in range(4):
                            nc.tensor.matmul(
                                ps[:, nb * 512:(nb + 1) * 512],
                                lhsT=wg2[:, kt, to * 128:(to + 1) * 128],
                                rhs=eT[:, kt, nb * 512:(nb + 1) * 512],
                                start=(kt == 0), stop=(kt == 3))
                    tmp = pool_tmp.tile([128, 1024], F16, name="g2t", tag="tmp")
                    nc.scalar.activation(out=tmp[:], in_=ps[:],
                                         func=AF.Identity, bias=bt2[:, to:to + 1])
                    nc.vector.tensor_add(h[:, to, :], tmp[:], xc[:, to, :])

                # FFN
                f1o = pool_b.tile([128, 8, N], F16, name="f1o")
                for to in range(8):
                    pp = psum if to % 2 == 0 else psums
                    ps = pp.tile([128, 1024], F32, name="ps_f1",
                                 tag="ps" if to % 2 == 0 else "psim")
                    for nb in range(2):
                        for kt in range(2):
                            nc.tensor.matmul(
                                ps[:, nb * 512:(nb + 1) * 512],
                                lhsT=wf1[:, kt, to * 128:(to + 1) * 128],
                                rhs=h[:, kt, nb * 512:(nb + 1) * 512],
                                start=(kt == 0), stop=(kt == 1))
                    if img == 0 or to % 2 == 0:
                        nc.scalar.activation(out=f1o[:, to, :], in_=ps[:],
                                             func=AF.Relu, bias=bbf1[:, to:to + 1])
                    else:
                        nc.vector.tensor_scalar(
                            out=f1o[:, to, :], in0=ps[:],
                            scalar1=bbf1[:, to:to + 1], scalar2=0.0,
                            op0=OP.add, op1=OP.max)
                h2 = pool_h2.tile([128, 2, N], F16, name="h2")
                for to in range(2):
                    pp = psum if to % 2 == 0 else psums
                    ps = pp.tile([128, 1024], F32, name="ps_f2",
                                 tag="ps" if to % 2 == 0 else "psim")
                    for nb in range(2):
                        for kt in range(8):
                            nc.tensor.matmul(
                                ps[:, nb * 512:(nb + 1) * 512],
                                lhsT=wf2[:, kt, to * 128:(to + 1) * 128],
                                rhs=f1o[:, kt, nb * 512:(nb + 1) * 512],
                                start=(kt == 0), stop=(kt == 7))
                    tmp = pool_tmp.tile([128, 1024], F16, name="f2t", tag="tmp")
                    nc.scalar.activation(out=tmp[:], in_=ps[:],
                                         func=AF.Identity, bias=bbf2[:, to:to + 1])
                    nc.vector.tensor_add(h2[:, to, :], tmp[:], h[:, to, :])
                hxs = pool_h2.tile([128, 2, N], F16, name="hxs")
                nc.vector.tensor_add(hxs[:], h2[:], xc[:])

                # bottleneck
                b1o = pool_b.tile([64, N], F16, name="b1o")
                ps1 = psum.tile([128, 1024], F32, name="ps_b1", tag="ps")
                for nb in range(2):
                    for kt in range(2):
                        nc.tensor.matmul(
                            ps1[0:64, nb * 512:(nb + 1) * 512], lhsT=wb1[:, kt, :],
                            rhs=h2[:, kt, nb * 512:(nb + 1) * 512],
                            start=(kt == 0), stop=(kt == 1))
                nc.scalar.activation(out=b1o[:], in_=ps1[0:64],
                                     func=AF.Relu, bias=btb1[0:64, 0:1])
                pad3 = pad[:].rearrange("p (r c) -> p r c", r=34)
                b1v = b1o[:].rearrange("p (r c) -> p r c", r=32)
                nc.vector.tensor_copy(pad3[:, 1:33, 1:33], b1v)
                b2o = pool_b.tile([64, N], F16, name="b2o")
                ps2 = psum.tile([128, 1024], F32, name="ps_b2", tag="ps")
                for nb in range(2):
                    for tap in range(9):
                        dy, dx = tap // 3, tap % 3
                        rhs = pad3[:, 16 * nb + dy:16 * nb + dy + 16, dx:dx + 32]
                        nc.tensor.matmul(ps2[0:64, nb * 512:(nb + 1) * 512],
                                         lhsT=wb2[0:64, tap, :], rhs=rhs,
                                         start=(tap == 0), stop=(tap == 8))
                nc.scalar.activation(out=b2o[:], in_=ps2[0:64],
                                     func=AF.Relu, bias=btb2[0:64, 0:1])

                # b3 + (h2+x) residual with final BN folded in
                ot = pool_e.tile([128, 2, N], F16, name="ot", tag="eT")
                odv = out_d[img].rearrange("(t p) n -> p t n", p=128)
                for to in range(2):
                    pp = psum if to % 2 == 0 else psums
                    ps = pp.tile([128, 1024], F32, name="ps_b3",
                                 tag="ps" if to % 2 == 0 else "psim")
                    for nb in range(2):
                        nc.tensor.matmul(
                            ps[:, nb * 512:(nb + 1) * 512],
                            lhsT=wb3[0:64, to * 128:(to + 1) * 128],
                            rhs=b2o[:, nb * 512:(nb + 1) * 512],
                            start=True, stop=True)
                    tmp = pool_tmp.tile([128, 1024], F16, name="b3t", tag="tmp")
                    nc.scalar.activation(out=tmp[:], in_=ps[:],
                                         func=AF.Identity, scale=bsf[:, to:to + 1],
                                         bias=b3b[:, to:to + 1])
                    nc.vector.scalar_tensor_tensor(
                        out=ot[:, to, :], in0=hxs[:, to, :],
                        scalar=bsf[:, to:to + 1], in1=tmp[:],
                        op0=OP.mult, op1=OP.add)
                    nc.sync.dma_start(out=odv[:, to], in_=ot[:, to, :])

            def g(im, s, h):
                return lambda: gather_s(im, s, h)

            def f(im, s, h):
                return lambda: fold_s(im, s, h)

            emit_F_front(0)
            emit_F_front(1)
            emit_F_main(0)
            emit_G_start(0)
            hooks0 = {0: [g(0, 0, 0), g(0, 0, 1)],
                      1: [g(0, 1, 0), g(0, 1, 1)],
                      2: [f(0, 0, 0), g(0, 2, 0)],
                      3: [f(0, 0, 1), g(0, 2, 1)],
                      4: [f(0, 1, 0), g(0, 3, 0)],
                      5: [f(0, 1, 1), g(0, 3, 1)],
                      6: [f(0, 2, 0), f(0, 2, 1)],
                      7: [f(0, 3, 0), f(0, 3, 1)]}
            emit_F_main(1, hooks=hooks0)
            emit_G_start(1)
            seq = [g(1, 0, 0), g(1, 0, 1), g(1, 1, 0), g(1, 1, 1),
                   f(1, 0, 0), g(1, 2, 0), f(1, 0, 1), g(1, 2, 1),
                   f(1, 1, 0), g(1, 3, 0), f(1, 1, 1), g(1, 3, 1),
                   f(1, 2, 0), f(1, 2, 1), f(1, 3, 0), f(1, 3, 1)]
            for fn in seq:
                fn()
            emit_B(0)
            emit_B(1)

    nc.finalize()
    return nc


def kernel(**inputs):
    inp = {k: np.asarray(v) for k, v in inputs.items()}
    w = _prep_weights(inp)

    if 'nc' not in _cache:
        _cache['nc'] = _build_bass()
    nc = _cache['nc']

    x = inp['x'].astype(np.float32).reshape(B, C, N)
    in_maps = []
    for c in range(N_CORES):
        m = {'x': np.ascontiguousarray(x[c * 2:(c + 1) * 2])}
        m.update(w)
        in_maps.append(m)

    from concourse.bass_utils import run_bass_kernel_spmd
    trace = bool(os.environ.get("KBENCH_TRACE"))
    res = run_bass_kernel_spmd(nc, in_maps, core_ids=list(range(N_CORES)),
                               trace=trace)
    _cache['exec_time_ns'] = res.exec_time_ns
    _cache['results'] = res
    out = np.zeros((B, C, N), np.float32)
    for c in range(N_CORES):
        out[c * 2:(c + 1) * 2] = res.results[c]['out'].astype(np.float32)
    return out.reshape(B, C, H, W)


# revision 49
# speedup vs baseline: 1.0006x; 1.0006x over previous
"""Trainium2 Bass kernel for nn_Block_17033840296551 (GNN message passing block).

Data-parallel over batch: 16 images -> 8 cores x 2 images. Software-pipelined
emission F0 F1 G0 B0 G1 B1 so the two images overlap (F = g1/norms/sim/topk/
q/p/idx-repack, G = gather+maxfold, B = g2/FFN/bottleneck/out).

Key points vs the straightforward version:
  * wrapped-idx columns laid out (g, k, i) per s-block so the top-k index
    repack is 8 contiguous per-g DMAs + 3 replication DMAs instead of ~39
    scattered ones; the gather output column order is reconciled with node
    order by strided eT/pT views in the fold tail (i-split, 3 free dims).
  * All weights in 3 DRAM params (f16 blob / f32 biases / skeleton) = 3 DMAs.
  * sim scores stay in PSUM; DVE Max/MaxIndex read PSUM directly (no evac).
  * EdgeConv gather runs from SBUF (q never round-trips DRAM).
  * PSUM evacuations are single fused ops (activation w/ bias, tensor_scalar
    relu(x+b), scalar_tensor_tensor (x+b)+res) spread across Act/DVE/Pool.
  * f16 residual stream; f32 only in PSUM and the final output.
"""

import os
import numpy as np

B, C, H, W = 16, 256, 32, 32
N = H * W
K = 9
EPS = 1e-5
IMGS_PER_CORE = 2
N_CORES = 8
NEG_BIG = -30000.0

_cache = {}

# f16 blob column offsets (per partition, f16 words)
_F16_OFF = {
    'wg1': (0, 512), 'wp': (512, 1024), 'wq': (1536, 1024), 'wg2': (2560, 1024),
    'wf1': (3584, 2048), 'wf2': (5632, 2048), 'wb1': (7680, 128),
    'wb2': (7808, 576), 'wb3': (8384, 256), 'negid': (8640, 128),
    'idbig2': (8768, 2048),
}
F16W = 10816
# f32 param columns
_F32_OFF = {'bt1': (0, 2), 'bt2': (2, 2), 'bbp': (4, 4), 'bbf1': (8, 8),
            'bbf2': (16, 2), 'btb3': (18, 2), 'bsf': (20, 2), 'btf': (22, 2),
            'btb1': (24, 1), 'btb2': (25, 1), 'nbbp': (26, 4),
            'b3b': (30, 2)}
F32W = 32


def _bn_fold(p):
    g, b, m, v = np.asarray(p, np.float32)
    s = g / np.sqrt(v + EPS)
    t = b - m * s
    return s, t


def _pack_kxm(w_t, part=128):
    Kd, M = w_t.shape
    kt = Kd // part
    return np.ascontiguousarray(w_t.reshape(kt, part, M).transpose(1, 0, 2))


def _pack_bias(b, part=128):
    n = b.shape[0]
    t = n // part
    return np.ascontiguousarray(b.reshape(t, part).T)  # [part, t]


def _make_skeleton():
    """wrapped-idx self columns: col = 144*s + 18*g + 2*0 + i holds node
    n_self = 256*s + 128*i + 16*g + p16 on partition p16 (the node whose
    neighbor list lives at ixbuf[16*g + p16, s, :, i])."""
    skel = np.zeros((16, 576), np.int16)
    for s in range(4):
        for g in range(8):
            for i in range(2):
                col = 144 * s + 18 * g + i
                for p16 in range(16):
                    skel[p16, col] = 256 * s + 128 * i + 16 * g + p16
    return skel


def _prep_weights(inp):
    f16 = np.float16
    s1, t1 = _bn_fold(inp['g1_bn'])
    Wg1 = s1[:, None] * inp['g1_w']
    s2, t2 = _bn_fold(inp['g2_bn'])
    Wg2 = s2[:, None] * inp['g2_w']
    sf1, tf1 = _bn_fold(inp['f1_bn'])
    Wf1 = sf1[:, None] * inp['f1_w']
    bf1 = sf1 * inp['f1_b'] + tf1
    sf2, tf2 = _bn_fold(inp['f2_bn'])
    Wf2 = sf2[:, None] * inp['f2_w']
    bf2 = sf2 * inp['f2_b'] + tf2
    sb1, tb1 = _bn_fold(inp['b1_bn'])
    Wb1 = sb1[:, None] * inp['b1_w']
    sb2, tb2 = _bn_fold(inp['b2_bn'])
    Wb2 = sb2[:, None, None, None] * inp['b2_w']
    sb3, tb3 = _bn_fold(inp['b3_bn'])
    Wb3 = sb3[:, None] * inp['b3_w']
    sf, tf = _bn_fold(inp['bnf'])

    A = inp['edge_w'][:, :C]
    Bm = inp['edge_w'][:, C:]
    Wp = A - Bm
    Wq = Bm
    bp = inp['edge_b']

    wb2_t = np.zeros((64, 9, 64), f16)
    for dy in range(3):
        for dx in range(3):
            wb2_t[:, dy * 3 + dx, :] = Wb2[:, :, dy, dx].T.astype(f16)

    f16b = np.zeros((128, F16W), f16)

    def put16(name, arr):
        off, n = _F16_OFF[name]
        p = arr.shape[0]
        f16b[:p, off:off + n] = arr.reshape(p, -1)

    put16('wg1', _pack_kxm(Wg1.T.astype(f16)))
    put16('wp', _pack_kxm(Wp.T.astype(f16)))
    put16('wq', _pack_kxm(Wq.T.astype(f16)))
    put16('wg2', _pack_kxm(Wg2.T.astype(f16)))
    put16('wf1', _pack_kxm(Wf1.T.astype(f16)))
    put16('wf2', _pack_kxm(Wf2.T.astype(f16)))
    put16('wb1', _pack_kxm(Wb1.T.astype(f16)))
    put16('wb2', wb2_t)
    put16('wb3', Wb3.T.astype(f16))
    put16('negid', (NEG_BIG * np.eye(128)).astype(f16))
    idbig2 = np.zeros((128, 2048), f16)
    for k in range(128):
        idbig2[k, 1024 + k] = 1.0
    put16('idbig2', idbig2)

    f32b = np.zeros((128, F32W), np.float32)

    def put32(name, arr):
        off, n = _F32_OFF[name]
        p = arr.shape[0]
        f32b[:p, off:off + n] = arr.reshape(p, -1)

    put32('bt1', _pack_bias(t1))
    put32('bt2', _pack_bias(t2))
    put32('bbp', _pack_bias(bp))
    put32('nbbp', _pack_bias(-bp))
    put32('bbf1', _pack_bias(bf1))
    put32('bbf2', _pack_bias(bf2))
    put32('btb3', _pack_bias(tb3))
    put32('bsf', _pack_bias(sf))
    put32('btf', _pack_bias(tf))
    put32('btb1', tb1[:, None].astype(np.float32))
    put32('btb2', tb2[:, None].astype(np.float32))
    put32('b3b', _pack_bias(sf * tb3 + tf))

    return {'wf16': f16b, 'wf32': f32b, 'skel': _make_skeleton()}


def _build_bass():
    import concourse.bass as bass
    import concourse.mybir as mybir
    from concourse import bacc
    from concourse.tile import TileContext

    dt = mybir.dt
    F16 = dt.float16
    F32 = dt.float32
    AF = mybir.ActivationFunctionType
    OP = mybir.AluOpType

    nc = bacc.Bacc()

    x_d = nc.declare_dram_parameter("x", [IMGS_PER_CORE, C, N], F32, isOutput=False)
    wf16_d = nc.declare_dram_parameter("wf16", [128, F16W], F16, isOutput=False)
    wf32_d = nc.declare_dram_parameter("wf32", [128, F32W], F32, isOutput=False)
    skel_d = nc.declare_dram_parameter("skel", [16, 576], dt.int16, isOutput=False)
    out_d = nc.declare_dram_parameter("out", [IMGS_PER_CORE, C, N], F16, isOutput=True)

    with TileContext(nc) as tc:
        import contextlib
        ctx = contextlib.ExitStack()
        with ctx:
            consts = ctx.enter_context(tc.tile_pool(name="consts", bufs=1))
            pool_x = ctx.enter_context(tc.tile_pool(name="x", bufs=2))
            pool_feat = ctx.enter_context(tc.tile_pool(name="feat", bufs=2))
            pool_nrm = ctx.enter_context(tc.tile_pool(name="nrm", bufs=2))
            pool_sm = ctx.enter_context(tc.tile_pool(name="sm", bufs=2))
            pool_mx = ctx.enter_context(tc.tile_pool(name="mx", bufs=4))
            pool_idx = ctx.enter_context(tc.tile_pool(name="idx", bufs=2))
            pool_qp = ctx.enter_context(tc.tile_pool(name="qp", bufs=2))
            pool_e = ctx.enter_context(tc.tile_pool(name="e", bufs=2))
            pool_go = ctx.enter_context(tc.tile_pool(name="go", bufs=4))
            pool_h = ctx.enter_context(tc.tile_pool(name="h", bufs=1))
            pool_h2 = ctx.enter_context(tc.tile_pool(name="h2", bufs=2))
            pool_tmp = ctx.enter_context(tc.tile_pool(name="tmp", bufs=4))
            pool_b = ctx.enter_context(tc.tile_pool(name="b", bufs=1))
            psum = ctx.enter_context(tc.tile_pool(name="psum", bufs=2, space="PSUM"))
            psums = ctx.enter_context(tc.tile_pool(name="psims", bufs=2, space="PSUM"))

            # ---- consts ----
            wf16 = consts.tile([128, F16W], F16, name="wf16")
            wf32 = consts.tile([128, F32W], F32, name="wf32")
            nc.sync.dma_start(out=wf16[:, 0:512], in_=wf16_d[:, 0:512])
            nc.sync.dma_start(out=wf32[:], in_=wf32_d[:])

            def w16(name, *shape):
                off, n = _F16_OFF[name]
                v = wf16[:, off:off + n]
                if len(shape) > 1:
                    v = v.rearrange("p (a b) -> p a b", a=shape[0])
                return v

            wg1 = w16('wg1', 2, 256)
            wp = w16('wp', 2, 512)
            wq = w16('wq', 2, 512)
            wg2 = w16('wg2', 4, 256)
            wf1 = w16('wf1', 2, 1024)
            wf2 = w16('wf2', 8, 256)
            wb1 = w16('wb1', 2, 64)
            wb2 = w16('wb2', 9, 64)
            wb3 = w16('wb3', 256)
            negid = w16('negid', 128)
            idbig2 = w16('idbig2', 2048)

            def b32(name):
                off, n = _F32_OFF[name]
                return wf32[:, off:off + n]

            bt1, bt2, bbp, bbf1 = b32('bt1'), b32('bt2'), b32('bbp'), b32('bbf1')
            bbf2, btb3, bsf, btf = b32('bbf2'), b32('btb3'), b32('bsf'), b32('btf')
            btb1, btb2, b3b = b32('btb1'), b32('btb2'), b32('b3b')

            ones = consts.tile([128, 128], F16, name="ones")
            nc.gpsimd.memset(ones[:], 1.0)
            warm = consts.tile([1, 8], F16, name="warm")
            nc.gpsimd.memset(warm[:], 1.0)
            nc.scalar.activation(out=warm[:], in_=warm[:], func=AF.Identity)
            nc.scalar.activation(out=warm[:], in_=warm[:], func=AF.Sqrt)

            # persistent wrapped-idx tiles (self cols filled once)
            wrapped = [consts.tile([128, 576], dt.int16, name=f"wr{i}")
                       for i in range(IMGS_PER_CORE)]
            for i in range(IMGS_PER_CORE):
                nc.sync.dma_start(out=wrapped[i][0:16, :], in_=skel_d[:])

            xcs, feats, xns, ixbufs, qsbs, pTs, eTs = {}, {}, {}, {}, {}, {}, {}
            houts = {}
            h2s = {}
            pads = {}

            # ================= F phase =================
            def emit_F_front(img):
                xc = pool_x.tile([128, 2, N], F16, name="xc")
                xcs[img] = xc
                nc.gpsimd.dma_start(
                    out=xc[:],
                    in_=x_d[img].rearrange("(t p) n -> p t n", p=128))
                if img == 0:
                    nc.sync.dma_start(out=wf16[:, 512:], in_=wf16_d[:, 512:])

                featT = pool_feat.tile([128, 2, N], F16, name="featT")
                feats[img] = featT
                for to in range(2):
                    ps = psum.tile([128, 1024], F32, name="ps_g1", tag="ps")
                    for nb in range(2):
                        for kt in range(2):
                            nc.tensor.matmul(
                                ps[:, nb * 512:(nb + 1) * 512],
                                lhsT=wg1[:, kt, to * 128:(to + 1) * 128],
                                rhs=xc[:, kt, nb * 512:(nb + 1) * 512],
                                start=(kt == 0), stop=(kt == 1))
                    nc.scalar.activation(out=featT[:, to, :], in_=ps[:],
                                         func=AF.Identity, bias=bt1[:, to:to + 1])

                # norms
                ve = nc.vector
                fsq = pool_nrm.tile([128, 2, N], F16, name="fsq")
                for t in range(2):
                    if img == 0:
                        nc.vector.tensor_mul(fsq[:, t, :], featT[:, t, :],
                                             featT[:, t, :])
                    else:
                        nc.scalar.activation(out=fsq[:, t, :],
                                             in_=featT[:, t, :], func=AF.Square)
                n2 = pool_sm.tile([1, N], F32, name="n2")
                for nb in range(2):
                    ps1 = psum.tile([128, 1024], F32, name="ps_n2", tag="ps")
                    for kt in range(2):
                        nc.tensor.matmul(
                            ps1[0:1, 0:512], lhsT=ones[:, 0:1],
                            rhs=fsq[:, kt, nb * 512:(nb + 1) * 512],
                            start=(kt == 0), stop=(kt == 1))
                    nc.scalar.activation(out=n2[:, nb * 512:(nb + 1) * 512],
                                         in_=ps1[0:1, 0:512], func=AF.Identity)
                nc.vector.reciprocal(out=n2[:], in_=n2[:])
                invn = pool_sm.tile([1, N], F16, name="invn")
                nc.scalar.activation(out=invn[:], in_=n2[:], func=AF.Sqrt)
                invnb = pool_nrm.tile([128, N], F16, name="invnb")
                ps = psum.tile([128, 1024], F32, name="ps_bc", tag="ps")
                for nb in range(2):
                    nc.tensor.matmul(ps[:, nb * 512:(nb + 1) * 512],
                                     lhsT=ones[0:1, :],
                                     rhs=invn[:, nb * 512:(nb + 1) * 512],
                                     start=True, stop=True)
                nc.scalar.activation(out=invnb[:], in_=ps[:], func=AF.Identity)
                xnT = pool_nrm.tile([128, 2, N], F16, name="xnT")
                for t in range(2):
                    ve.tensor_mul(xnT[:, t, :], featT[:, t, :], invnb[:])
                xns[img] = xnT

            def emit_F_main(img, hooks=None):
                featT = feats[img]
                xnT = xns[img]
                # sim + top-8 (Max/MaxIndex read PSUM directly)
                ixbuf = pool_idx.tile([128, 4, 8, 2], dt.uint16, name="ixbuf")
                ixbufs[img] = ixbuf
                for I in range(8):
                    ps = psums.tile([128, 1024], F32, name="ps_sim", tag="psim")
                    for cb in range(2):
                        has_diag = (cb == I // 4)
                        for kt in range(2):
                            nc.tensor.matmul(
                                ps[:, cb * 512:(cb + 1) * 512],
                                lhsT=featT[:, kt, I * 128:(I + 1) * 128],
                                rhs=xnT[:, kt, cb * 512:(cb + 1) * 512],
                                start=(kt == 0),
                                stop=(kt == 1 and not has_diag))
                        if has_diag:
                            Wo = 1024 - 128 * (I % 4)
                            nc.tensor.matmul(
                                ps[:, cb * 512:(cb + 1) * 512],
                                lhsT=negid,
                                rhs=idbig2[:, Wo:Wo + 512],
                                start=False, stop=True)
                    mx = pool_mx.tile([128, 8], F32, name="mx")
                    nc.vector.max(out=mx[:], in_=ps[:])
                    nc.vector.max_index(out=ixbuf[:, I // 2, :, I % 2],
                                        in_max=mx[:], in_values=ps[:])
                    for fn in (hooks or {}).get(I, []):
                        fn()

                # idx repack: 8 contiguous fold DMAs + 3 replication DMAs
                wr = wrapped[img]
                wrv = wr.rearrange("p (s c) -> p s c", s=4)
                ixi = ixbuf[:].bitcast(dt.int16)
                for g in range(8):
                    nc.sync.dma_start(
                        out=wrv[0:16, :, 18 * g + 2:18 * g + 18],
                        in_=ixi[16 * g:16 * (g + 1)].rearrange("p s k i -> p s (k i)"))
                nc.sync.dma_start(out=wr[16:32, :], in_=wr[0:16, :])
                nc.sync.dma_start(out=wr[32:64, :], in_=wr[0:32, :])
                nc.sync.dma_start(out=wr[64:128, :], in_=wr[0:64, :])

                # q (pairs of nt into one [128,1024] psum)
                q_sb = pool_qp.tile([128, 8, 512], F16, name="q_sb")
                qsbs[img] = q_sb
                for np_ in range(4):
                    ps = psum.tile([128, 1024], F32, name="ps_q", tag="ps")
                    for half in range(2):
                        nt = 2 * np_ + half
                        for kt in range(2):
                            nc.tensor.matmul(
                                ps[:, half * 512:(half + 1) * 512],
                                lhsT=featT[:, kt, nt * 128:(nt + 1) * 128],
                                rhs=wq[:, kt, :], start=(kt == 0), stop=(kt == 1))
                    nc.scalar.activation(
                        out=q_sb[:].rearrange("p a b -> p (a b)")[:, np_ * 1024:(np_ + 1) * 1024],
                        in_=ps[:], func=AF.Identity)

                # p^T (bias folded at evac)
                pT = pool_qp.tile([128, 4, N], F16, name="pT")
                pTs[img] = pT
                for to in range(4):
                    ps = psum.tile([128, 1024], F32, name="ps_p", tag="ps")
                    for nb in range(2):
                        for kt in range(2):
                            nc.tensor.matmul(
                                ps[:, nb * 512:(nb + 1) * 512],
                                lhsT=wp[:, kt, to * 128:(to + 1) * 128],
                                rhs=featT[:, kt, nb * 512:(nb + 1) * 512],
                                start=(kt == 0), stop=(kt == 1))
                    nc.scalar.activation(out=pT[:, to, :], in_=ps[:],
                                         func=AF.Identity, bias=bbp[:, to:to + 1])

            # ================= G phase =================
            gos = {}

            def emit_G_start(img):
                eTs[img] = pool_e.tile([128, 4, N], F16, name="eT")

            def gather_s(img, s, h):
                q_sb = qsbs[img]
                wr = wrapped[img]
                go = pool_go.tile([128, 4, 1152], F16, name="go")
                gos[(img, s, h)] = go
                nc.gpsimd.dma_gather(
                    out_ap=go[:], in_ap=q_sb[:].rearrange("p a b -> p (a b)"),
                    idxs_ap=wr[:, 144 * s + 72 * h:144 * s + 72 * (h + 1)],
                    num_idxs=1152, num_idxs_reg=1152, elem_size=512,
                    transpose=True, sbuf_tokens_per_rank=128,
                    sbuf_free_dim_per_rank=1024,
                    single_packet=False)

            def fold_s(img, s, h):
                go = gos.pop((img, s, h))
                pT = pTs[img]
                eT = eTs[img]
                # go cols: j = 288g' + 32k + 16i + p16, g' in 0..3 (g = 4h+g')
                gv = go[:].rearrange("p a (g k w) -> p (a g) k w", g=4, k=9)
                nc.vector.tensor_max(gv[:, :, 5:9, :], gv[:, :, 1:5, :],
                                     gv[:, :, 5:9, :])
                nc.vector.tensor_max(gv[:, :, 7:9, :], gv[:, :, 5:7, :],
                                     gv[:, :, 7:9, :])
                nc.vector.tensor_max(gv[:, :, 8, :], gv[:, :, 7, :],
                                     gv[:, :, 8, :])
                # tail: output (g', i, q) <-> node 256s + 128i + 16(4h+g') + q
                g6 = go[:].rearrange("p a (g k i q) -> p a g k i q",
                                     g=4, k=9, i=2)
                for i in range(2):
                    base = 256 * s + 128 * i + 64 * h
                    ev = eT[:, :, base:base + 64].rearrange(
                        "p a (g q) -> p a g q", g=4)
                    pv = pT[:, :, base:base + 64].rearrange(
                        "p a (g q) -> p a g q", g=4)
                    nc.vector.tensor_max(ev, g6[:, :, :, 8, i, :],
                                         g6[:, :, :, 0, i, :])
                    nc.vector.tensor_add(ev, ev, pv)
                    nc.scalar.activation(out=eT[:, :, base:base + 64],
                                         in_=eT[:, :, base:base + 64],
                                         func=AF.Relu)

            # ================= B phase =================
            def emit_B_ffn(img):
                xc = xcs[img]
                eT = eTs[img]
                # g2 + residual -> h f16
                pad = pool_b.tile([64, 34 * 34], F16, name="pad")
                pads[img] = pad
                nc.gpsimd.memset(pad[:], 0.0)
                h = pool_h.tile([128, 2, N], F16, name="h")
                for to in range(2):
                    pp = psum if to % 2 == 0 else psums
                    ps = pp.tile([128, 1024], F32, name="ps_g2",
                                 tag="ps" if to % 2 == 0 else "psim")
                    for nb in range(2):
                        for kt in range(4):
                            nc.tensor.matmul(
                                ps[:, nb * 512:(nb + 1) * 512],
                                lhsT=wg2[:, kt, to * 128:(to + 1) * 128],
                                rhs=eT[:, kt, nb * 512:(nb + 1) * 512],
                                start=(kt == 0), stop=(kt == 3))
                    tmp = pool_tmp.tile([128, 1024], F16, name="g2t", tag="tmp")
                    nc.scalar.activation(out=tmp[:], in_=ps[:],
                                         func=AF.Identity, bias=bt2[:, to:to + 1])
                    nc.vector.tensor_add(h[:, to, :], tmp[:], xc[:, to, :])

                # FFN
                f1o = pool_b.tile([128, 8, N], F16, name="f1o")
                for to in range(8):
                    pp = psum if to % 2 == 0 else psums
                    ps = pp.tile([128, 1024], F32, name="ps_f1",
                                 tag="ps" if to % 2 == 0 else "psim")
                    for nb in range(2):
                        for kt in range(2):
                            nc.tensor.matmul(
                                ps[:, nb * 512:(nb + 1) * 512],
                                lhsT=wf1[:, kt, to * 128:(to + 1) * 128],
                                rhs=h[:, kt, nb * 512:(nb + 1) * 512],
                                start=(kt == 0), stop=(kt == 1))
                    if img == 0 or to % 2 == 0:
                        nc.scalar.activation(out=f1o[:, to, :], in_=ps[:],
                                             func=AF.Relu, bias=bbf1[:, to:to + 1])
                    else:
                        nc.vector.tensor_scalar(
                            out=f1o[:, to, :], in0=ps[:],
                            scalar1=bbf1[:, to:to + 1], scalar2=0.0,
                            op0=OP.add, op1=OP.max)
                h2 = pool_h2.tile([128, 2, N], F16, name="h2")
                h2s[img] = h2
                for to in range(2):
                    pp = psum if to % 2 == 0 else psums
                    ps = pp.tile([128, 1024], F32, name="ps_f2",
                                 tag="ps" if to % 2 == 0 else "psim")
                    for nb in range(2):
                        for kt in range(8):
                            nc.tensor.matmul(
                                ps[:, nb * 512:(nb + 1) * 512],
                                lhsT=wf2[:, kt, to * 128:(to + 1) * 128],
                                rhs=f1o[:, kt, nb * 512:(nb + 1) * 512],
                                start=(kt == 0), stop=(kt == 7))
                    tmp = pool_tmp.tile([128, 1024], F16, name="f2t", tag="tmp")
                    nc.scalar.activation(out=tmp[:], in_=ps[:],
                                         func=AF.Identity, bias=bbf2[:, to:to + 1])
                    nc.vector.tensor_add(h2[:, to, :], tmp[:], h[:, to, :])
            def emit_B_neck(img):
                xc = xcs[img]
                h2 = h2s[img]
                pad = pads[img]
                hxs = pool_h2.tile([128, 2, N], F16, name="hxs")
                nc.vector.tensor_add(hxs[:], h2[:], xc[:])

                # bottleneck
                b1o = pool_b.tile([64, N], F16, name="b1o")
                ps1 = psum.tile([128, 1024], F32, name="ps_b1", tag="ps")
                for nb in range(2):
                    for kt in range(2):
                        nc.tensor.matmul(
                            ps1[0:64, nb * 512:(nb + 1) * 512], lhsT=wb1[:, kt, :],
                            rhs=h2[:, kt, nb * 512:(nb + 1) * 512],
                            start=(kt == 0), stop=(kt == 1))
                nc.scalar.activation(out=b1o[:], in_=ps1[0:64],
                                     func=AF.Relu, bias=btb1[0:64, 0:1])
                pad3 = pad[:].rearrange("p (r c) -> p r c", r=34)
                b1v = b1o[:].rearrange("p (r c) -> p r c", r=32)
                nc.vector.tensor_copy(pad3[:, 1:33, 1:33], b1v)
                b2o = pool_b.tile([64, N], F16, name="b2o")
                ps2 = psum.tile([128, 1024], F32, name="ps_b2", tag="ps")
                for nb in range(2):
                    for tap in range(9):
                        dy, dx = tap // 3, tap % 3
                        rhs = pad3[:, 16 * nb + dy:16 * nb + dy + 16, dx:dx + 32]
                        nc.tensor.matmul(ps2[0:64, nb * 512:(nb + 1) * 512],
                                         lhsT=wb2[0:64, tap, :], rhs=rhs,
                                         start=(tap == 0), stop=(tap == 8))
                nc.scalar.activation(out=b2o[:], in_=ps2[0:64],
                                     func=AF.Relu, bias=btb2[0:64, 0:1])

                # b3 + (h2+x) residual with final BN folded in
                ot = pool_e.tile([128, 2, N], F16, name="ot", tag="eT")
                odv = out_d[img].rearrange("(t p) n -> p t n", p=128)
                for to in range(2):
                    pp = psum if to % 2 == 0 else psums
                    ps = pp.tile([128, 1024], F32, name="ps_b3",
                                 tag="ps" if to % 2 == 0 else "psim")
                    for nb in range(2):
                        nc.tensor.matmul(
                            ps[:, nb * 512:(nb + 1) * 512],
                            lhsT=wb3[0:64, to * 128:(to + 1) * 128],
                            rhs=b2o[:, nb * 512:(nb + 1) * 512],
                            start=True, stop=True)
                    tmp = pool_tmp.tile([128, 1024], F16, name="b3t", tag="tmp")
                    nc.scalar.activation(out=tmp[:], in_=ps[:],
                                         func=AF.Identity, scale=bsf[:, to:to + 1],
                                         bias=b3b[:, to:to + 1])
                    nc.vector.scalar_tensor_tensor(
                        out=ot[:, to, :], in0=hxs[:, to, :],
                        scalar=bsf[:, to:to + 1], in1=tmp[:],
                        op0=OP.mult, op1=OP.add)
                    nc.sync.dma_start(out=odv[:, to], in_=ot[:, to, :])

            def g(im, s, h):
                return lambda: gather_s(im, s, h)

            def f(im, s, h):
                return lambda: fold_s(im, s, h)

            emit_F_front(0)
            emit_F_front(1)
            emit_F_main(0)
            emit_G_start(0)
            hooks0 = {0: [g(0, 0, 0), g(0, 0, 1)],
                      1: [g(0, 1, 0), g(0, 1, 1)],
                      2: [f(0, 0, 0), g(0, 2, 0)],
                      3: [f(0, 0, 1), g(0, 2, 1)],
                      4: [f(0, 1, 0), g(0, 3, 0)],
                      5: [f(0, 1, 1), g(0, 3, 1)],
                      6: [f(0, 2, 0), f(0, 2, 1)],
                      7: [f(0, 3, 0), f(0, 3, 1)]}
            emit_F_main(1, hooks=hooks0)
            emit_G_start(1)
            seq = [g(1, 0, 0), g(1, 0, 1), g(1, 1, 0), g(1, 1, 1),
                   f(1, 0, 0), g(1, 2, 0), f(1, 0, 1), g(1, 2, 1),
                   lambda: emit_B_ffn(0),
                   f(1, 1, 0), g(1, 3, 0), f(1, 1, 1), g(1, 3, 1),
                   f(1, 2, 0), f(1, 2, 1),
                   lambda: emit_B_neck(0),
                   f(1, 3, 0), f(1, 3, 1)]
            for fn in seq:
                fn()
            emit_B_ffn(1)
            emit_B_neck(1)

    nc.finalize()
    return nc


def kernel(**inputs):
    inp = {k: np.asarray(v) for k, v in inputs.items()}
    w = _prep_weights(inp)

    if 'nc' not in _cache:
        _cache['nc'] = _build_bass()
    nc = _cache['nc']

    x = inp['x'].astype(np.float32).reshape(B, C, N)
    in_maps = []
    for c in range(N_CORES):
        m = {'x': np.ascontiguousarray(x[c * 2:(c + 1) * 2])}
        m.update(w)
        in_maps.append(m)

    from concourse.bass_utils import run_bass_kernel_spmd
    trace = bool(os.environ.get("KBENCH_TRACE"))
    res = run_bass_kernel_spmd(nc, in_maps, core_ids=list(range(N_CORES)),
                               trace=trace)
    _cache['exec_time_ns'] = res.exec_time_ns
    _cache['results'] = res
    out = np.zeros((B, C, N), np.float32)
    for c in range(N_CORES):
        out[c * 2:(c + 1) * 2] = res.results[c]['out'].astype(np.float32)
    return out.reshape(B, C, H, W)
